# revision 37
# baseline (speedup 1.0000x reference)
"""Multi-head attention (B=4, S=2048, D=1024, H=16) on 8 Trainium2 cores.

Sharding: core c handles batch b=c//2 and head-group g=c%2 (8 heads, 512 of
the 1024 head dims).  Each core computes q/k/v projections for its head
slice, causal attention for its 8 heads, and a partial output projection
(contraction over its 512 concat dims).  The host sums the two partials per
batch and adds the dense bias.  No on-device collectives needed.

Per-core kernel (bf16 matmuls, fp32 accumulation):
  - PE-transpose activations 128x128 blocks (fp32), cast to bf16 in the
    PSUM->SBUF copy; project with host-pre-transposed bf16 weight slices.
  - Phase order: k proj, v proj, then per 512-row sq-chunk: q proj for the
    chunk, flash attention for all 8 heads, partial dense for the chunk —
    so ScalarE exp work overlaps PE projection work.
  - Flash attention per (head, chunk): logitsT [sk, sq] blocks, exp on
    ScalarE (scale folded), diagonal-block causal mask multiply, AV matmul
    accumulating outT_aug [65, sq] in PSUM (row 64 = exp row sums).
  - Row-sum normalization: broadcast via a rank-1 PE matmul (ones x rs),
    reciprocal on DVE, multiply into concatT.
  - Dense: concatT @ dense_wT -> partial output [S, D].
"""

import ml_dtypes
import numpy as np

import concourse.bass as bass
from concourse import bacc
import concourse.mybir as mybir
import concourse.tile as tile
from concourse.bass_utils import run_bass_kernel_spmd
from concourse.masks import make_identity, make_upper_triangular

B, S, D, H = 4, 2048, 1024, 16
DEPTH = 64
HPC = 8          # heads per core
DH = HPC * DEPTH  # 512: per-core head width
N_CORES = 8
SCALE = 1.0 / 32.0  # 1/sqrt(D)
NEG = -1e9 * 32.0   # mask bias, pre-divided by SCALE

FR = mybir.dt.float32r
F32 = mybir.dt.float32
BF = mybir.dt.bfloat16

Exp = mybir.ActivationFunctionType.Exp
Copy = mybir.ActivationFunctionType.Copy
Ident = mybir.ActivationFunctionType.Identity

NT = S // 128    # 16 sequence tiles of 128
NCH = S // 512   # 4 sequence chunks of 512
KT = D // 128    # 8 contraction tiles for the projections
CH = 512         # row-chunk for the input transpose pipeline


def _build(variant: str, with_bias: bool) -> bass.Bass:
    """variant: 'causal' (triu mask), 'full' (no mask), 'general' (additive)."""
    nc = bacc.Bacc()

    xqT = nc.declare_dram_parameter("xqT", [D, S], BF, isOutput=False).ap()
    xkT = nc.declare_dram_parameter("xkT", [D, S], BF, isOutput=False).ap()
    xvT = nc.declare_dram_parameter("xvT", [D, S], BF, isOutput=False).ap()
    wqT = nc.declare_dram_parameter("wqT", [D, DH], BF, isOutput=False).ap()
    wkT = nc.declare_dram_parameter("wkT", [D, DH], BF, isOutput=False).ap()
    wvT = nc.declare_dram_parameter("wvT", [D, DH], BF, isOutput=False).ap()
    dwT = nc.declare_dram_parameter("dwT", [DH, D], BF, isOutput=False).ap()
    if with_bias:
        qb = nc.declare_dram_parameter("qb", [DH], F32, isOutput=False).ap()
        kb = nc.declare_dram_parameter("kb", [DH], F32, isOutput=False).ap()
        vb = nc.declare_dram_parameter("vb", [DH], F32, isOutput=False).ap()
    if variant == "general":
        # mask.T pre-scaled by -1e9/SCALE, [sk, sq]
        mT = nc.declare_dram_parameter("mT", [S, S], F32, isOutput=False).ap()
    outp = nc.declare_dram_parameter("outp", [S, D], F32, isOutput=True).ap()

    with tile.TileContext(nc) as tc:
        with (
            tc.tile_pool(name="const", bufs=1) as const,
            tc.tile_pool(name="wpool", bufs=1) as wpool,

            tc.tile_pool(name="xp", bufs=5) as x_pool,
            tc.tile_pool(name="ptp", bufs=6) as pt_pool,
            tc.tile_pool(name="nrm", bufs=4) as nrm_pool,
            tc.tile_pool(name="mskp", bufs=2) as msk_pool,
            tc.tile_pool(name="otp", bufs=3) as ot_pool,
            # PSUM budget (8 banks): p=2, lg=2x2banks, av=2
            tc.tile_pool(name="pjs", bufs=2, space="PSUM") as p_psum,
            tc.tile_pool(name="lgs", bufs=2, space="PSUM") as lg_psum,
            tc.tile_pool(name="avs", bufs=2, space="PSUM") as av_psum,
            tc.tile_pool(name="drs", bufs=4, space="DRAM") as dr_pool,
        ):
            if variant == "causal":
                binm = const.tile([128, 128], BF)
                # binm[r, q] = 1 if q >= r else 0 (keep sq >= sk in diag block)
                make_upper_triangular(nc, binm, val=1.0, diag=True)
            qhT = const.tile([128, DH // 128, S], BF)
            khT = const.tile([128, DH // 128, S], BF)
            vha = const.tile([128, NT, HPC, DEPTH + 1], BF)
            nc.vector.memset(vha[:, :, :, DEPTH], 1.0)
            catT = const.tile([128, DH // 128, S], BF)

            if with_bias:
                qb_sb = const.tile([128, DH // 128], F32)
                nc.sync.dma_start(out=qb_sb,
                                  in_=qb.rearrange("(m p) -> p m", p=128))
                kb_sb = const.tile([128, DH // 128], F32)
                nc.sync.dma_start(out=kb_sb,
                                  in_=kb.rearrange("(m p) -> p m", p=128))
                vb_bc = const.tile([128, HPC, DEPTH], F32)
                nc.sync.dma_start(
                    out=vb_bc,
                    in_=vb.rearrange("(h d) -> h d", h=HPC)
                    .unsqueeze(0)
                    .partition_broadcast(128),
                )

            def load_x(xT_dram, c, nm):
                """one CH-chunk of a transposed bf16 input -> [128, KT, CH]"""
                xs = x_pool.tile([128, KT, CH], BF, tag="x", name=nm)
                nc.sync.dma_start(
                    out=xs,
                    in_=xT_dram[:, c * CH : (c + 1) * CH].rearrange(
                        "(t p) s -> p t s", p=128),
                )
                return xs

            def proj_T(x_sb, wt, dst, bsb, c):
                """qhT/khT-style projection of one CH-chunk into dst."""
                for m in range(DH // 128):
                    ps = p_psum.tile([128, CH], F32, tag="pj")
                    for kt in range(KT):
                        nc.tensor.matmul(
                            ps,
                            lhsT=wt[:, kt, 128 * m : 128 * (m + 1)],
                            rhs=x_sb[:, kt, :],
                            start=(kt == 0),
                            stop=(kt == KT - 1),
                        )
                    if with_bias:
                        nc.scalar.activation(
                            dst[:, m, c * CH : (c + 1) * CH], ps, Ident,
                            bias=bsb[:, m : m + 1],
                        )
                    else:
                        nc.scalar.activation(
                            dst[:, m, c * CH : (c + 1) * CH], ps, Copy)

            def proj_v(x_sb, wt, c):
                """v projection of one CH-chunk into vha (natural layout)."""
                for t in range(CH // 128):
                    j = (c * CH) // 128 + t
                    ps = p_psum.tile([128, DH], F32, tag="pj")
                    for kt in range(KT):
                        nc.tensor.matmul(
                            ps,
                            lhsT=x_sb[:, kt, 128 * t : 128 * (t + 1)],
                            rhs=wt[:, kt, :],
                            start=(kt == 0),
                            stop=(kt == KT - 1),
                        )
                    psv = ps.rearrange("p (h d) -> p h d", h=HPC)
                    if with_bias:
                        nc.vector.tensor_add(vha[:, j, :, 0:DEPTH], psv, vb_bc)
                    else:
                        nc.vector.tensor_copy(vha[:, j, :, 0:DEPTH], psv)

            def attention_pair(h0, c, filler=None, rate=0.0):
                """Two heads (h0, h0+1) interleaved per j-block so the PE
                never stalls on ScalarE exp, and the odd head's lhsT sits in
                row groups 2-3 (LDWEIGHTS overlap with the even head).
                After each j-block, ~rate steps of `filler` (independent PE
                work) are emitted to keep the PE dense while ScalarE runs."""
                heads = (h0, h0 + 1)
                fill_acc = 0.0
                jmax = 4 * c + 3 if variant == "causal" else NT - 1
                avs = {}
                for h in heads:
                    avs[h] = av_psum.tile([65, 512], F32, tag="av",
                                          name=f"av_{h}_{c}")
                for j in range(jmax + 1):
                    t = j - 4 * c
                    off = 128 * t if (variant == "causal" and t >= 0) else 0
                    mblk = None
                    if variant == "general":
                        mblk = msk_pool.tile([128, 512], F32, tag="mb")
                        nc.sync.dma_start(
                            out=mblk,
                            in_=mT[128 * j : 128 * (j + 1),
                                   512 * c : 512 * (c + 1)],
                        )
                    lg2 = lg_psum.tile([128, 1024], F32, tag="lg",
                                       name=f"lg2_{c}_{j}")
                    pt2 = pt_pool.tile([128, 1024], BF, tag="pt",
                                       name=f"pt2_{c}_{j}")
                    pts = {}
                    for i, h in enumerate(heads):
                        p0 = 64 * (h % 2)
                        lo = 512 * i
                        nc.tensor.matmul(
                            lg2[:, lo + off : lo + 512],
                            lhsT=khT[p0 : p0 + 64, h // 2,
                                     128 * j : 128 * (j + 1)],
                            rhs=qhT[p0 : p0 + 64, h // 2,
                                    512 * c + off : 512 * (c + 1)],
                            start=True,
                            stop=True,
                        )
                        if mblk is not None:
                            nc.vector.tensor_add(lg2[:, lo : lo + 512],
                                                 lg2[:, lo : lo + 512], mblk)
                        pts[h] = pt2[:, lo : lo + 512]
                    if variant == "causal" and t >= 0:
                        # diagonal block: exp the two valid halves separately
                        # and apply the triangular mask
                        for i, h in enumerate(heads):
                            lo = 512 * i
                            nc.scalar.activation(
                                pt2[:, lo + off : lo + 512],
                                lg2[:, lo + off : lo + 512], Exp, scale=SCALE)
                            nc.gpsimd.tensor_mul(
                                pt2[:, lo + off : lo + off + 128],
                                pt2[:, lo + off : lo + off + 128], binm)
                    else:
                        nc.scalar.activation(pt2, lg2, Exp, scale=SCALE)
                    if filler is not None:
                        fill_acc += rate
                        while fill_acc >= 1.0:
                            fill_acc -= 1.0
                            if next(filler, None) is StopIteration:
                                break
                    for h in heads:
                        nc.tensor.matmul(
                            avs[h][:, off:],
                            lhsT=vha[:, j, h, :],
                            rhs=pts[h][:, off:] if off else pts[h],
                            start=(j == 0),
                            stop=(j == jmax),
                        )
                # normalize by exp row sums (row 64 of av). Copy av out of
                # PSUM fast (releases the bank), then a PE-free DRAM-roundtrip
                # broadcast of the sums, reciprocal, multiply into concatT.
                for h in reversed(heads):
                    p0 = 64 * (h % 2)
                    mt = h // 2
                    av_sb = nrm_pool.tile([65, 512], F32, tag="avs",
                                          name=f"avsb_{h}_{c}")
                    nc.scalar.copy(av_sb, avs[h])
                    rs_dr = dr_pool.tile([512], F32, tag="rsd")
                    nc.sync.dma_start(out=rs_dr, in_=av_sb[64:65, :])
                    rb = nrm_pool.tile([64, 512], F32, tag="rb")
                    nc.sync.dma_start(
                        out=rb, in_=rs_dr.unsqueeze(0).partition_broadcast(64))
                    nc.vector.reciprocal(rb, rb)
                    if p0 == 0:
                        nc.vector.tensor_mul(
                            catT[0:64, mt, 512 * c : 512 * (c + 1)],
                            av_sb[0:64, :], rb)
                    else:
                        bnc = nrm_pool.tile([64, 512], BF, tag="bnc")
                        nc.vector.tensor_mul(bnc, av_sb[0:64, :], rb)
                        nc.sync.dma_start(
                            out=catT[64:128, mt, 512 * c : 512 * (c + 1)],
                            in_=bnc)

            def dense_steps(wt, c, kts=None, accum=False):
                """partial output projection for sq-chunk c, one matmul per
                yield — interleaved into attention j-loops as PE filler.
                kts: subset of contraction tiles (split passes; the second
                pass accumulates into DRAM via the DMA compute engine)."""
                if kts is None:
                    kts = list(range(DH // 128))
                for st in range(4 * c, 4 * (c + 1)):
                    for oc in range(D // 512):
                        ps = p_psum.tile([128, 512], F32, tag="pj",
                                         name=f"dps_{st}_{oc}")
                        for i, kt in enumerate(kts):
                            nc.tensor.matmul(
                                ps,
                                lhsT=catT[:, kt, 128 * st : 128 * (st + 1)],
                                rhs=wt[:, kt, 512 * oc : 512 * (oc + 1)],
                                start=(i == 0),
                                stop=(i == len(kts) - 1),
                            )
                            yield
                        ob = ot_pool.tile([128, 512], F32, tag="ob",
                                          name=f"ob_{st}_{oc}")
                        nc.vector.tensor_copy(ob, ps)
                        if accum:
                            nc.gpsimd.dma_start(
                                out=outp[128 * st : 128 * (st + 1),
                                         512 * oc : 512 * (oc + 1)],
                                in_=ob, accum_op=mybir.AluOpType.add)
                        else:
                            nc.sync.dma_start(
                                out=outp[128 * st : 128 * (st + 1),
                                         512 * oc : 512 * (oc + 1)],
                                in_=ob)
                        yield

            def dense(wt, c):
                for _ in dense_steps(wt, c):
                    pass

            # ---- k and v projections ----
            xk0 = x_pool.tile([128, KT, CH], BF, tag="x", name="xk0pre")
            for kt in range(KT):
                nc.sync.dma_start(out=xk0[:, kt, :],
                                  in_=xkT[128 * kt : 128 * (kt + 1), 0:CH])
            wk_sb = wpool.tile([128, KT, DH], BF, tag="wk")
            nc.sync.dma_start(out=wk_sb,
                              in_=wkT.rearrange("(t p) n -> p t n", p=128))
            wv_sb = wpool.tile([128, KT, DH], BF, tag="wv")
            nc.sync.dma_start(out=wv_sb,
                              in_=wvT.rearrange("(t p) n -> p t n", p=128))
            wq_sb = wpool.tile([128, KT, DH], BF, tag="wq")
            nc.sync.dma_start(out=wq_sb,
                              in_=wqT.rearrange("(t p) n -> p t n", p=128))
            wd_sb = wpool.tile([128, DH // 128, D], BF, tag="wd")
            nc.sync.dma_start(out=wd_sb,
                              in_=dwT.rearrange("(t p) n -> p t n", p=128))

            if variant == "causal":
                # causal: attention chunk c only needs k/v rows < 512(c+1),
                # so k/v/q projections interleave with attention per chunk
                for cq in range(NCH):
                    xs = xk0 if cq == 0 else load_x(xkT, cq, f"xk{cq}")
                    proj_T(xs, wk_sb, khT, kb_sb if with_bias else None, cq)
                    xs = load_x(xvT, cq, f"xv{cq}")
                    proj_v(xs, wv_sb, cq)
                    xs = load_x(xqT, cq, f"xq{cq}")
                    proj_T(xs, wq_sb, qhT, qb_sb if with_bias else None, cq)
                    filler = dense_steps(wd_sb, cq - 1) if cq > 0 else None
                    nsteps = 4 * (DH // 128 + 1) * (D // 512)
                    njs = (HPC // 2) * (4 * cq + 4)
                    njs = njs * 3 // 4  # no filler during the first pair
                    rate = (nsteps / njs + 0.01) if filler is not None else 0.0
                    for h0 in range(0, HPC, 2):
                        attention_pair(h0, cq,
                                       filler if h0 >= 2 else None, rate)
                    if filler is not None:
                        for _ in filler:  # drain any remainder
                            pass
                dense(wd_sb, NCH - 1)
            else:
                # full/general: every attention chunk reads all of k/v
                for cq in range(NCH):
                    xs = xk0 if cq == 0 else load_x(xkT, cq, f"xk{cq}")
                    proj_T(xs, wk_sb, khT, kb_sb if with_bias else None, cq)
                    xs = load_x(xvT, cq, f"xv{cq}")
                    proj_v(xs, wv_sb, cq)
                for cq in range(NCH):
                    xs = load_x(xqT, cq, f"xq{cq}")
                    proj_T(xs, wq_sb, qhT, qb_sb if with_bias else None, cq)
                    for h0 in range(0, HPC, 2):
                        attention_pair(h0, cq)
                    dense(wd_sb, cq)

    nc.compile()
    return nc


_CACHE: dict = {}


def _get_nc(variant: str, with_bias: bool) -> bass.Bass:
    key = (variant, with_bias)
    if key not in _CACHE:
        _CACHE[key] = _build(variant, with_bias)
    return _CACHE[key]


def _make_in_maps(q, k, v, mask2d, wq_w, wk_w, wv_w, dense_w,
                  wq_b, wk_b, wv_b, variant, with_bias):
    xT_cache = {}
    for b in range(B):
        xT_cache[b] = tuple(
            np.ascontiguousarray(x[b].T.astype(ml_dtypes.bfloat16))
            for x in (q, k, v))
    in_maps = []
    for core in range(N_CORES):
        b, g = divmod(core, 2)
        rows = slice(DH * g, DH * (g + 1))
        im = {
            "xqT": xT_cache[b][0],
            "xkT": xT_cache[b][1],
            "xvT": xT_cache[b][2],
            "wqT": np.ascontiguousarray(wq_w[rows].T.astype(ml_dtypes.bfloat16)),
            "wkT": np.ascontiguousarray(wk_w[rows].T.astype(ml_dtypes.bfloat16)),
            "wvT": np.ascontiguousarray(wv_w[rows].T.astype(ml_dtypes.bfloat16)),
            "dwT": np.ascontiguousarray(
                dense_w[:, rows].T.astype(ml_dtypes.bfloat16)),
        }
        if with_bias:
            im["qb"] = np.ascontiguousarray(wq_b[rows])
            im["kb"] = np.ascontiguousarray(wk_b[rows])
            im["vb"] = np.ascontiguousarray(wv_b[rows])
        if variant == "general":
            im["mT"] = np.ascontiguousarray(mask2d.T * np.float32(NEG))
        in_maps.append(im)
    return in_maps


def kernel(q, k, v, mask, wq_w, wq_b, wk_w, wk_b, wv_w, wv_b,
           dense_w, dense_b, **run_kwargs):
    q = np.asarray(q, np.float32)
    k = np.asarray(k, np.float32)
    v = np.asarray(v, np.float32)
    mask2d = np.asarray(mask, np.float32).reshape(S, S)
    wq_w = np.asarray(wq_w, np.float32)
    wk_w = np.asarray(wk_w, np.float32)
    wv_w = np.asarray(wv_w, np.float32)
    dense_w = np.asarray(dense_w, np.float32)
    wq_b = np.asarray(wq_b, np.float32)
    wk_b = np.asarray(wk_b, np.float32)
    wv_b = np.asarray(wv_b, np.float32)
    dense_b = np.asarray(dense_b, np.float32)

    causal_ref = np.triu(np.ones((S, S), np.float32), k=1)
    if np.array_equal(mask2d, causal_ref):
        variant = "causal"
    elif not mask2d.any():
        variant = "full"
    else:
        variant = "general"
    with_bias = bool(wq_b.any() or wk_b.any() or wv_b.any())

    nc = _get_nc(variant, with_bias)
    in_maps = _make_in_maps(q, k, v, mask2d, wq_w, wk_w, wv_w, dense_w,
                            wq_b, wk_b, wv_b, variant, with_bias)
    res = run_bass_kernel_spmd(nc, in_maps, core_ids=list(range(N_CORES)),
                               **run_kwargs)
    outs = res.results
    out = np.empty((B, S, D), np.float32)
    for b in range(B):
        out[b] = outs[2 * b]["outp"] + outs[2 * b + 1]["outp"]
    out += dense_b[None, None, :].astype(np.float32)
    globals()["_last_results"] = res
    return out
